# revision 5
# baseline (speedup 1.0000x reference)
"""Trainium2 Bass kernel for nn_ConcatRelationModule (gnn_message_passing).

Strategy: shard edges by HEAD WINDOW across 8 NeuronCores so each core only
needs fwd rows from its own 32768-token window; those are deduplicated
(~63% unique) and shipped compacted as int8 with per-row scales. The
per-edge expansion happens on-device with dma_gather. dma_gather requires
256-byte elements, so the compacted int8 table is stored as ROW PAIRS
([U/2, 256]); edges are grouped by parity of their compacted row index
(even-parity slots in the first tiles, odd in the rest), each tile reading
the first or second 128-byte half of its gathered pair elements. bwd rows
(mods = e+1) are host-gathered into slot order, int8. The axon-tunneled
PJRT path moves ~50MB/s, so wire bytes dominate wall time; inputs are
merged into 3 parameters (int8 blob / fp16 blob / int16 indices) to dodge
per-argument dispatch overhead.
 - per 512-edge tile: dequant int8 (per-row scale is per-partition in
   slot-major layout) + upcast to f32 on ScalarE, PE-transpose to
   [feature, edge], 3-layer MLP on the PE (fp32r), tanh/bias on ScalarE,
   margin (best-wrong minus gold label score) on VectorE.
 - raw margins dtile = wrong - gold return as fp16; host scatters them to
   edge order, applies the hinge, and re-evaluates exactly (fp32 numpy) the
   few edges within DELTA of the hinge boundary, where int8 quantization
   could flip the branch.
"""
import sys

sys.path.insert(0, "/opt/trn_rl_repo")

import numpy as np

import concourse.bass as bass
import concourse.bacc as bacc
import concourse.mybir as mybir
import concourse.tile as tile
from concourse.bass_utils import run_bass_kernel_spmd
from concourse.masks import make_identity

F32 = mybir.dt.float32
F32R = mybir.dt.float32r
F16 = mybir.dt.float16
I8 = mybir.dt.int8
I16 = mybir.dt.int16

N = 262144
L = 128
H = 128
H2 = 128
R = 64
E = N - 1
NCORES = 8
WIN = N // NCORES            # token window per core (head-sharded)
PAR_CAP = 16896              # slots per parity group (33*512; max parity count must fit)
G_CAP = 2 * PAR_CAP          # 33792 slots, 66 tiles
U_CAP = 20864                # unique fwd rows capacity (max observed 20804)
NB = G_CAP // 128            # 272 blocks
NT = G_CAP // 512            # 68 tiles
PAR_TILES = PAR_CAP // 512   # 33 tiles per parity group
# gather chunks within one parity group (sum = PAR_CAP)
GATHER_CHUNKS = [2048] * 8 + [512]

SZ_FWDU = U_CAP * L                  # 2752512
SZ_BWDS = G_CAP * L                  # 4456448
SZ_D8 = SZ_FWDU + SZ_BWDS + G_CAP    # + rels
SZ_W = 3 * 256 * 128 + 128 * R       # 106496 fp16 weights
SZ_B = 2 * H + H2 + R                # 448 fp16 biases
SZ_A16 = SZ_W + SZ_B + 3 * G_CAP     # + fwsc + bwsc + idx (int16 bits)

MM_MODE = "f32r"

DELTA = 0.15  # hinge-boundary band recomputed exactly on host


def build_kernel(mode=MM_MODE):
    mmdt = F32 if mode == "f32" else F32R
    nc = bacc.Bacc("TRN2", target_bir_lowering=False, debug=False)

    data8_d = nc.declare_dram_parameter("data8", [SZ_D8], I8, isOutput=False)
    aux16_d = nc.declare_dram_parameter("aux16", [SZ_A16], F16, isOutput=False)
    idx_d = aux16_d[SZ_W + SZ_B + 2 * G_CAP:SZ_A16].bitcast(I16).rearrange(
        "(p f) -> p f", f=G_CAP // 16)

    fwdu_pairs = data8_d[0:SZ_FWDU].rearrange("(u e) -> u e", e=256)
    bwd_rows = data8_d[SZ_FWDU:SZ_FWDU + SZ_BWDS].rearrange(
        "(t j p f) -> t p j f", p=128, j=4, f=L)
    rels_ap = data8_d[SZ_FWDU + SZ_BWDS:].rearrange("(p b) -> p b", b=NB)

    wslc = lambda a, b, f: aux16_d[a:b].rearrange("(p f) -> p f", f=f)
    bias_ap = lambda a, b: aux16_d[SZ_W + a:SZ_W + b].rearrange("(p o) -> p o", o=1)
    fwsc_ap = wslc(SZ_W + SZ_B, SZ_W + SZ_B + G_CAP, NB)
    bwsc_ap = wslc(SZ_W + SZ_B + G_CAP, SZ_W + SZ_B + 2 * G_CAP, NB)

    lerr_d = nc.declare_dram_parameter("dtile", [G_CAP], F16, isOutput=True)
    lerr_v = lerr_d[:].rearrange("(b p) -> b p", p=128)

    with tile.TileContext(nc) as tc:
        with (
            tc.tile_pool(name="const", bufs=1) as cp,
            tc.tile_pool(name="gath", bufs=4) as gp,
            tc.tile_pool(name="work", bufs=2) as wp,
            tc.tile_pool(name="ps", bufs=1, space="PSUM") as pp,
            tc.tile_pool(name="ps2", bufs=2, space="PSUM") as pp2,
        ):
            # ---- constants ----
            ident = cp.tile([128, 128], F32, tag="ident")
            make_identity(nc, ident[:])
            if mode == "f32":
                ident_m = ident
            else:
                ident_m = cp.tile([128, 128], F32R, tag="ident_m")
                nc.vector.tensor_copy(out=ident_m[:], in_=ident[:])

            wfoh_f = cp.tile([128, H], mmdt, tag="wfoh_f")
            wfoh_b = cp.tile([128, H], mmdt, tag="wfoh_b")
            wfom_f = cp.tile([128, H], mmdt, tag="wfom_f")
            wfom_b = cp.tile([128, H], mmdt, tag="wfom_b")
            rh2_a = cp.tile([128, H2], mmdt, tag="rh2_a")
            rh2_b = cp.tile([128, H2], mmdt, tag="rh2_b")
            rout_t = cp.tile([128, R], mmdt, tag="rout_t")
            for i, dst in enumerate((wfoh_f, wfoh_b, wfom_f, wfom_b, rh2_a, rh2_b)):
                wtmp = wp.tile([128, 128], F16, tag="wtmp")
                nc.sync.dma_start(out=wtmp[:], in_=wslc(i * 16384, (i + 1) * 16384, 128))
                nc.scalar.copy(out=dst[:], in_=wtmp[:])
            wtmp64 = wp.tile([128, R], F16, tag="wtmp64")
            nc.sync.dma_start(out=wtmp64[:], in_=wslc(98304, SZ_W, R))
            nc.scalar.copy(out=rout_t[:], in_=wtmp64[:])

            bias_h = cp.tile([128, 1], F32, tag="bias_h")
            bias_m = cp.tile([128, 1], F32, tag="bias_m")
            bias_2 = cp.tile([128, 1], F32, tag="bias_2")
            bias_r = cp.tile([64, 1], F32, tag="bias_r")
            for dst, a, b in ((bias_h, 0, 128), (bias_m, 128, 256),
                              (bias_2, 256, 384), (bias_r, 384, 448)):
                btmp = wp.tile([b - a, 1], F16, tag="btmp")
                nc.sync.dma_start(out=btmp[:], in_=bias_ap(a, b))
                nc.scalar.copy(out=dst[:], in_=btmp[:])

            iota_t = cp.tile([128, 4 * R], F32, tag="iota")
            nc.gpsimd.iota(
                out=iota_t[:].rearrange("p (j r) -> p j r", r=R),
                pattern=[[0, 4], [1, R]],
                channel_multiplier=0,
                allow_small_or_imprecise_dtypes=True,
            )

            rels8 = cp.tile([128, NB], I8, tag="rels8")
            nc.sync.dma_start(out=rels8[:], in_=rels_ap)
            rels_sb = cp.tile([128, NB], F32, tag="rels_sb")
            nc.scalar.copy(out=rels_sb[:], in_=rels8[:])
            fwsc16 = cp.tile([128, NB], F16, tag="fwsc16")
            nc.sync.dma_start(out=fwsc16[:], in_=fwsc_ap)
            fwsc_sb = cp.tile([128, NB], F32, tag="fwsc_sb")
            nc.scalar.copy(out=fwsc_sb[:], in_=fwsc16[:])
            bwsc16 = cp.tile([128, NB], F16, tag="bwsc16")
            nc.sync.dma_start(out=bwsc16[:], in_=bwsc_ap)
            bwsc_sb = cp.tile([128, NB], F32, tag="bwsc_sb")
            nc.scalar.copy(out=bwsc_sb[:], in_=bwsc16[:])

            # gather indices: [16, G/16] in DRAM, replicated to 128 partitions
            idx_sb = cp.tile([128, G_CAP // 16], I16, tag="idx_sb")
            for k in range(8):
                nc.sync.dma_start(out=idx_sb[16 * k:16 * (k + 1), :], in_=idx_d)

            lerr_acc = cp.tile([128, NB], F32, tag="lerr_acc")

            # ---- main pipeline ----
            t_global = 0
            for par in range(2):
                slot = par * PAR_CAP
                for gn in GATHER_CHUNKS:
                    # gather pair-elements: slot s -> partition s%128,
                    # 256 bytes at col (s//128)*256
                    fwd8g = gp.tile([128, 4096], I8, tag="fwd8g")
                    nc.gpsimd.dma_gather(
                        out_ap=fwd8g[:, 0:gn * 2].rearrange("p (j e) -> p j e", e=256),
                        in_ap=fwdu_pairs,
                        idxs_ap=idx_sb[:, slot // 16:(slot + gn) // 16],
                        num_idxs=gn,
                        num_idxs_reg=gn,
                        elem_size=256,
                        single_packet=False,
                    )
                    for ti in range(gn // 512):
                        t = t_global
                        off = ti * 512

                        bwd8 = wp.tile([128, 512], I8, tag="bwd8")
                        nc.sync.dma_start(
                            out=bwd8[:].rearrange("p (j f) -> p j f", f=128),
                            in_=bwd_rows[t],
                        )
                        # dequant: per-row scale is per-partition here; fwd
                        # reads the par-half of each gathered 256B pair
                        fwd_sb = wp.tile([128, 512], mmdt, tag="fwd_sb")
                        bwd_sb = wp.tile([128, 512], mmdt, tag="bwd_sb")
                        for j in range(4):
                            b = 4 * t + j
                            fcol = (off + j * 128) * 2 + par * 128
                            nc.scalar.activation(
                                out=fwd_sb[:, j * 128:(j + 1) * 128],
                                in_=fwd8g[:, fcol:fcol + 128],
                                func=mybir.ActivationFunctionType.Copy,
                                scale=fwsc_sb[:, b:b + 1],
                            )
                            nc.scalar.activation(
                                out=bwd_sb[:, j * 128:(j + 1) * 128],
                                in_=bwd8[:, j * 128:(j + 1) * 128],
                                func=mybir.ActivationFunctionType.Copy,
                                scale=bwsc_sb[:, b:b + 1],
                            )

                        # transpose to [feature, edge]
                        tp = pp2.tile([128, 512], mmdt, tag="tp")
                        for k in range(4):
                            nc.tensor.transpose(
                                out=tp[:, k * 128:(k + 1) * 128],
                                in_=fwd_sb[:, k * 128:(k + 1) * 128],
                                identity=ident_m[:],
                            )
                        fwdT = wp.tile([128, 512], mmdt, tag="fwdT")
                        nc.scalar.copy(out=fwdT[:], in_=tp[:])

                        tpb = pp.tile([128, 512], mmdt, tag="tpb")
                        for k in range(4):
                            nc.tensor.transpose(
                                out=tpb[:, k * 128:(k + 1) * 128],
                                in_=bwd_sb[:, k * 128:(k + 1) * 128],
                                identity=ident_m[:],
                            )
                        bwdT = wp.tile([128, 512], mmdt, tag="bwdT")
                        nc.scalar.copy(out=bwdT[:], in_=tpb[:])

                        fov = pp.tile([128, 512], F32, tag="fov")
                        nc.tensor.matmul(out=fov[:], lhsT=wfoh_f[:], rhs=fwdT[:],
                                         start=True, stop=False)
                        nc.tensor.matmul(out=fov[:], lhsT=wfoh_b[:], rhs=bwdT[:],
                                         start=False, stop=True)
                        h1 = wp.tile([128, 512], mmdt, tag="h1")
                        nc.scalar.activation(
                            out=h1[:], in_=fov[:],
                            func=mybir.ActivationFunctionType.Tanh,
                            bias=bias_h[:, 0:1],
                        )

                        mov = pp.tile([128, 512], F32, tag="mov")
                        nc.tensor.matmul(out=mov[:], lhsT=wfom_f[:], rhs=fwdT[:],
                                         start=True, stop=False)
                        nc.tensor.matmul(out=mov[:], lhsT=wfom_b[:], rhs=bwdT[:],
                                         start=False, stop=True)
                        h1m = wp.tile([128, 512], mmdt, tag="h1m")
                        nc.scalar.activation(
                            out=h1m[:], in_=mov[:],
                            func=mybir.ActivationFunctionType.Tanh,
                            bias=bias_m[:, 0:1],
                        )

                        h2p = pp.tile([128, 512], F32, tag="h2p")
                        nc.tensor.matmul(out=h2p[:], lhsT=rh2_a[:], rhs=h1[:],
                                         start=True, stop=False)
                        nc.tensor.matmul(out=h2p[:], lhsT=rh2_b[:], rhs=h1m[:],
                                         start=False, stop=True)
                        h2s = wp.tile([128, 512], mmdt, tag="h2s")
                        nc.scalar.activation(
                            out=h2s[:], in_=h2p[:],
                            func=mybir.ActivationFunctionType.Tanh,
                            bias=bias_2[:, 0:1],
                        )

                        scp = pp.tile([64, 512], F32, tag="scp")
                        nc.tensor.matmul(out=scp[:], lhsT=rout_t[:], rhs=h2s[:],
                                         start=True, stop=True)
                        ssb = wp.tile([64, 512], F32, tag="ssb")
                        nc.scalar.activation(
                            out=ssb[:], in_=scp[:],
                            func=mybir.ActivationFunctionType.Identity,
                            bias=bias_r[:, 0:1],
                        )

                        # scores back to [edge, label] layout
                        stp = pp.tile([128, 4 * R], F32, tag="stp")
                        for k in range(4):
                            nc.tensor.transpose(
                                out=stp[:, k * R:(k + 1) * R],
                                in_=ssb[:, k * 128:(k + 1) * 128],
                                identity=ident[0:64, 0:64],
                            )
                        st3 = stp[:].rearrange("p (j r) -> p j r", r=R)

                        # hinge margin on VectorE
                        relx = rels_sb[:, 4 * t:4 * t + 4].to_broadcast([128, 4, R])
                        mask = wp.tile([128, 4 * R], F32, tag="mask")
                        nc.vector.tensor_tensor(
                            out=mask[:].rearrange("p (j r) -> p j r", r=R),
                            in0=iota_t[:].rearrange("p (j r) -> p j r", r=R),
                            in1=relx,
                            op=mybir.AluOpType.is_equal,
                        )
                        m3 = mask[:].rearrange("p (j r) -> p j r", r=R)
                        gmul = wp.tile([128, 4 * R], F32, tag="gmul")
                        nc.vector.tensor_tensor(
                            out=gmul[:].rearrange("p (j r) -> p j r", r=R),
                            in0=st3, in1=m3, op=mybir.AluOpType.mult,
                        )
                        gold = wp.tile([128, 4], F32, tag="gold")
                        nc.vector.reduce_sum(
                            out=gold[:], in_=gmul[:].rearrange("p (j r) -> p j r", r=R),
                            axis=mybir.AxisListType.X,
                        )
                        wm = wp.tile([128, 4 * R], F32, tag="wm")
                        nc.vector.scalar_tensor_tensor(
                            out=wm[:].rearrange("p (j r) -> p j r", r=R),
                            in0=m3, scalar=-1e30, in1=st3,
                            op0=mybir.AluOpType.mult, op1=mybir.AluOpType.add,
                        )
                        wrong = wp.tile([128, 4], F32, tag="wrong")
                        nc.vector.reduce_max(
                            out=wrong[:], in_=wm[:].rearrange("p (j r) -> p j r", r=R),
                            axis=mybir.AxisListType.X,
                        )
                        nc.vector.tensor_tensor(
                            out=lerr_acc[:, 4 * t:4 * t + 4],
                            in0=wrong[:], in1=gold[:],
                            op=mybir.AluOpType.subtract,
                        )
                        t_global += 1
                    slot += gn

            # ---- write out margins (transpose to slot-major) ----
            for a in range(0, NB, 128):
                cols = min(128, NB - a)
                otp = pp2.tile([128, 128], F32, tag="tp")
                nc.tensor.transpose(
                    out=otp[0:cols, :],
                    in_=lerr_acc[:, a:a + cols],
                    identity=ident[:],
                )
                osb = wp.tile([128, 128], F16, tag="osb")
                nc.scalar.copy(out=osb[0:cols, :], in_=otp[0:cols, :])
                nc.sync.dma_start(out=lerr_v[a:a + cols, :], in_=osb[0:cols, :])

    nc.compile()
    return nc


_NC_CACHE = {}


def _get_nc(mode):
    if mode not in _NC_CACHE:
        _NC_CACHE[mode] = build_kernel(mode)
    return _NC_CACHE[mode]


def prepare_core_inputs(fwd, bwd, gold_heads, gold_rels, weights):
    """Head-window sharding with compacted unique fwd rows.

    Core c gets the edges whose head lies in [c*WIN, (c+1)*WIN). Slots are
    ordered [even-parity compacted row index | odd-parity], each group
    padded to PAR_CAP. Returns (in_maps, slot_edges).
    """
    def quant8(x):
        x = np.asarray(x, dtype=np.float32)
        s = np.abs(x).max(axis=1) / 127.0
        s[s == 0] = 1.0
        q = np.rint(x * (1.0 / s)[:, None]).astype(np.int8)
        return q, s.astype(np.float32)

    fwd8, fws = quant8(fwd)
    bwd8, bws = quant8(bwd)
    heads = np.asarray(gold_heads, dtype=np.int64)
    rels = np.asarray(gold_rels, dtype=np.int64)

    def sc_tile16(s):
        return s.reshape(NB, 128).T.astype(np.float16).reshape(-1)

    aux_w = np.concatenate([
        np.asarray(weights["wfoh"], np.float16).reshape(-1),
        np.asarray(weights["wfom"], np.float16).reshape(-1),
        np.asarray(weights["rh2"], np.float16).reshape(-1),
        np.asarray(weights["rout"], np.float16).reshape(-1),
        np.asarray(weights["bcat"], np.float16).reshape(-1),
        np.asarray(weights["b2"], np.float16).reshape(-1),
        np.asarray(weights["bout"], np.float16).reshape(-1),
    ])
    assert len(aux_w) == SZ_W + SZ_B

    in_maps = []
    slot_edges_all = []
    for c in range(NCORES):
        edges_c = np.nonzero((heads >> 15) == c)[0]
        h_loc = (heads[edges_c] & (WIN - 1)).astype(np.int64)
        uniq, inv = np.unique(h_loc, return_inverse=True)
        n_u = len(uniq)
        if n_u > U_CAP:
            raise OverflowError("unique fwd rows overflow")

        slot_edges = np.full(G_CAP, -1, dtype=np.int64)
        idx_slot = np.zeros(G_CAP, dtype=np.int16)
        fwsc = np.ones(G_CAP, dtype=np.float32)
        for par in range(2):
            sel = (inv & 1) == par
            k = int(sel.sum())
            if k > PAR_CAP:
                raise OverflowError("parity group overflow")
            base = par * PAR_CAP
            slot_edges[base:base + k] = edges_c[sel]
            idx_slot[base:base + k] = (inv[sel] >> 1).astype(np.int16)
            fwsc[base:base + k] = fws[c * WIN + h_loc[sel]]
        idx16 = np.ascontiguousarray(idx_slot.reshape(G_CAP // 16, 16).T)

        valid = slot_edges >= 0
        mods = np.where(valid, slot_edges + 1, 0)
        bwds = bwd8[mods]
        bwds[~valid] = 0
        bwsc = np.where(valid, bws[mods], 1.0).astype(np.float32)
        rels_slot = np.where(valid, rels[np.where(valid, slot_edges, 0)], 0)
        rels_arr = rels_slot.astype(np.uint8).reshape(NB, 128).T

        fwdu = np.zeros((U_CAP, L), dtype=np.int8)
        fwdu[:n_u] = fwd8[c * WIN + uniq]

        data8 = np.empty(SZ_D8, dtype=np.int8)
        data8[0:SZ_FWDU] = fwdu.reshape(-1)
        data8[SZ_FWDU:SZ_FWDU + SZ_BWDS] = bwds.reshape(-1)
        data8[SZ_FWDU + SZ_BWDS:] = np.ascontiguousarray(rels_arr).reshape(-1).view(np.int8)
        aux16 = np.empty(SZ_A16, dtype=np.float16)
        aux16[0:SZ_W + SZ_B] = aux_w
        aux16[SZ_W + SZ_B:SZ_W + SZ_B + G_CAP] = sc_tile16(fwsc)
        aux16[SZ_W + SZ_B + G_CAP:SZ_W + SZ_B + 2 * G_CAP] = sc_tile16(bwsc)
        aux16[SZ_W + SZ_B + 2 * G_CAP:] = idx16.reshape(-1).view(np.float16)

        in_maps.append(dict(data8=data8, aux16=aux16))
        slot_edges_all.append(slot_edges)
    return in_maps, slot_edges_all


def _exact_lerr(edges, fwd, bwd, gold_heads, gold_rels, WFOH, WFOM, rcatBias,
                rhid2Layer, rhid2Bias, routLayer, routBias):
    """fp32 numpy re-evaluation of the reference for a subset of edges."""
    heads = np.asarray(gold_heads, dtype=np.int64)[edges]
    rels = np.asarray(gold_rels, dtype=np.int64)[edges]
    cat = np.concatenate(
        [np.asarray(fwd, np.float32)[heads], np.asarray(bwd, np.float32)[edges + 1]],
        axis=-1,
    )
    h = np.tanh(
        np.concatenate([cat @ WFOH, cat @ WFOM], axis=-1)
        + np.asarray(rcatBias, np.float32).reshape(1, -1)
    )
    h2 = np.tanh(h @ rhid2Layer + np.asarray(rhid2Bias, np.float32).reshape(1, -1))
    scores = h2 @ routLayer + np.asarray(routBias, np.float32).reshape(1, -1)
    k = len(edges)
    gold_s = scores[np.arange(k), rels]
    scores[np.arange(k), rels] = -np.inf
    wrong_s = scores.max(axis=1)
    return np.where(gold_s < wrong_s + 1.0, wrong_s - gold_s, 0.0).astype(np.float32)


def assemble_output(results, slot_edges_all, fwd, bwd, gold_heads, gold_rels,
                    WFOH, WFOM, rcatBias, rhid2Layer, rhid2Bias, routLayer,
                    routBias):
    dt = np.zeros(E, dtype=np.float32)
    for c in range(NCORES):
        out = np.asarray(results[c]["dtile"], dtype=np.float32)
        se = slot_edges_all[c]
        valid = se >= 0
        dt[se[valid]] = out[valid]
    lerr = np.where(dt > -1.0, dt, 0.0).astype(np.float32)
    band = np.nonzero(np.abs(dt + 1.0) < DELTA)[0]
    if len(band):
        lerr[band] = _exact_lerr(
            band, fwd, bwd, gold_heads, gold_rels, WFOH, WFOM, rcatBias,
            rhid2Layer, rhid2Bias, routLayer, routBias,
        )
    return lerr


def kernel(fwd, bwd, gold_heads, gold_rels, WFOH, WFOM, rhidBias, rcatBias,
           rhid2Layer, rhid2Bias, routLayer, routBias):
    nc = _get_nc(MM_MODE)
    WFOH = np.ascontiguousarray(WFOH, dtype=np.float32)
    WFOM = np.ascontiguousarray(WFOM, dtype=np.float32)
    rhid2Layer = np.ascontiguousarray(rhid2Layer, dtype=np.float32)
    routLayer = np.ascontiguousarray(routLayer, dtype=np.float32)
    weights = dict(
        wfoh=WFOH.astype(np.float16),
        wfom=WFOM.astype(np.float16),
        rh2=rhid2Layer.astype(np.float16),
        rout=routLayer.astype(np.float16),
        bcat=np.asarray(rcatBias, dtype=np.float32).reshape(-1).astype(np.float16),
        b2=np.asarray(rhid2Bias, dtype=np.float32).reshape(-1).astype(np.float16),
        bout=np.asarray(routBias, dtype=np.float32).reshape(-1).astype(np.float16),
    )
    in_maps, slot_edges_all = prepare_core_inputs(
        fwd, bwd, gold_heads, gold_rels, weights)
    res = run_bass_kernel_spmd(nc, in_maps, list(range(NCORES)))
    return assemble_output(
        res.results, slot_edges_all, fwd, bwd, gold_heads, gold_rels, WFOH,
        WFOM, rcatBias, rhid2Layer, rhid2Bias, routLayer, routBias,
    )
